# revision 11
# baseline (speedup 1.0000x reference)
# GQA attention block on 8 Trainium2 NeuronCores — restructured bf16 edition.
# Sharding: core = (batch b in {0,1}) x (tensor-parallel t in {0..3}).
# Each core: batch row b, 4 query heads {4t..4t+3}, 2 kv heads {2t, 2t+1}.
# W_Q/W_K/W_V split column-wise (per-head), W_O row-wise; the 4 TP partial
# outputs per batch are summed on the host (the "all-reduce").
#
# vs the naive schedule:
#  - softmax exp batched over PSUM bank-pairs (N=1024 per ACT instruction)
#  - softmax denominator moved off the tensor engine: DVE running adds over
#    the ex tiles + one gpsimd partition_all_reduce (PE saves a full second
#    pass over ex), reciprocal via the fast DVE approximation
#  - output projection interleaved per 512-row block so PE never drains
#  - RoPE in bf16 (2x DVE modes), output written as bf16 (halves out DMA)
import math
import sys

sys.path.insert(0, "/opt/trn_rl_repo")

import ml_dtypes
import numpy as np

import concourse.bacc as bacc
import concourse.bass as bass
import concourse.bass_isa as bass_isa
import concourse.mybir as mybir
import concourse.tile as tile
from contextlib import ExitStack

BF = mybir.dt.bfloat16
F32 = mybir.dt.float32
bfnp = ml_dtypes.bfloat16

EMB = 2048
HEADS = 16
G = 2
HD = 128          # head dim
KV = HEADS // G   # 8 kv heads
B = 2
S = 2048
NCORES = 8
TP = 4
HQ = HEADS // TP       # 4 q heads per core
HKV = KV // TP         # 2 kv heads per core
NE = EMB // 128        # 16 contraction chunks
SC4 = S // 512         # 4 s-chunks of 512
SC16 = S // 128        # 16 s-chunks of 128
SCALE = 1.0 / math.sqrt(float(EMB))

_NC = None


def _build_program(loop_n=None):
    nc = bacc.Bacc("TRN2", target_bir_lowering=False, debug=False)

    xT = nc.dram_tensor("xT", (EMB, S), BF, kind="ExternalInput")
    wq = nc.dram_tensor("wq", (EMB, HQ * HD), BF, kind="ExternalInput")
    wk = nc.dram_tensor("wk", (EMB, HKV * HD), BF, kind="ExternalInput")
    wv = nc.dram_tensor("wv", (EMB, HKV * HD), BF, kind="ExternalInput")
    wo = nc.dram_tensor("wo", (HQ * HD, EMB), BF, kind="ExternalInput")
    cosT = nc.dram_tensor("cosT", (HD, S), BF, kind="ExternalInput")
    sinT = nc.dram_tensor("sinT", (HD, S), BF, kind="ExternalInput")
    out = nc.dram_tensor("out", (S, EMB), BF, kind="ExternalOutput")

    with tile.TileContext(nc) as tc, ExitStack() as ctx:
        persist = ctx.enter_context(tc.tile_pool(name="persist", bufs=1))
        # roped Q (jb 0..3) and K (jb 4..5), bf16: [d, jb, sc, s512]
        qk_sb = persist.tile([128, HQ + HKV, SC4, 512], BF)
        # V in [t, d] layout: [t_part, t_chunk, kvl*128+d]
        v_sb = persist.tile([128, SC16, HKV * HD], BF)
        ctx_sb = persist.tile([128, HQ, SC4, 512], BF)   # [d, head, sc, s]
        wo_sb = persist.tile([128, HQ, SC4, 512], BF)    # [d, head, ec, e]
        xt_sb = persist.tile([128, NE, S], BF)
        wqs = persist.tile([128, NE, HQ * HD], BF)
        wks = persist.tile([128, NE, HKV * HD], BF)
        wvs = persist.tile([128, NE, HKV * HD], BF)
        cos_sb = persist.tile([128, SC4, 512], BF)
        sin_sb = persist.tile([128, SC4, 512], BF)

        # batched input loads: few multi-dim DMAs (the SP sequencer pays
        # ~0.6us dispatch per DMA). xT is split so its completion semaphores
        # fire progressively and the first projection can start early.
        xTr = xT.rearrange("(c p) s -> p c s", p=128)
        wkr = wk.rearrange("(c p) j -> p c j", p=128)
        nc.sync.dma_start(out=xt_sb[:, 0:1, :], in_=xTr[:, 0:1, :])
        nc.sync.dma_start(out=wks[:, 0:4, :], in_=wkr[:, 0:4, :])
        nc.sync.dma_start(out=xt_sb[:, 1:2, :], in_=xTr[:, 1:2, :])
        nc.sync.dma_start(out=wks[:, 4:8, :], in_=wkr[:, 4:8, :])
        nc.sync.dma_start(out=wks[:, 8:16, :], in_=wkr[:, 8:16, :])
        nc.sync.dma_start(out=xt_sb[:, 2:4, :], in_=xTr[:, 2:4, :])
        for ci in range(2, 6):
            nc.sync.dma_start(
                out=xt_sb[:, 2 * ci:2 * ci + 2, :], in_=xTr[:, 2 * ci:2 * ci + 2, :]
            )
        nc.sync.dma_start(out=wvs, in_=wv.rearrange("(c p) j -> p c j", p=128))
        for ci in range(6, 8):
            nc.sync.dma_start(
                out=xt_sb[:, 2 * ci:2 * ci + 2, :], in_=xTr[:, 2 * ci:2 * ci + 2, :]
            )
        nc.sync.dma_start(out=wqs, in_=wq.rearrange("(c p) j -> p c j", p=128))
        nc.sync.dma_start(out=cos_sb, in_=cosT.rearrange("p (sc s) -> p sc s", s=512))
        nc.sync.dma_start(out=sin_sb, in_=sinT.rearrange("p (sc s) -> p sc s", s=512))
        nc.sync.dma_start(
            out=wo_sb, in_=wo.rearrange("(jb p) (ec e) -> p jb ec e", p=128, e=512)
        )

        # PSUM budget (8 banks): pairs 2x2 + accp 2 + oacc 2
        pairs = ctx.enter_context(tc.tile_pool(name="pairs", bufs=2, space="PSUM"))
        accp = ctx.enter_context(tc.tile_pool(name="accp", bufs=2, space="PSUM"))
        oacc = ctx.enter_context(tc.tile_pool(name="oacc", bufs=2, space="PSUM"))
        ropet = ctx.enter_context(tc.tile_pool(name="ropet", bufs=3))
        expool = ctx.enter_context(tc.tile_pool(name="expool", bufs=3))
        dccp = ctx.enter_context(tc.tile_pool(name="dccp", bufs=2))
        darp = ctx.enter_context(tc.tile_pool(name="darp", bufs=2))
        rbp = ctx.enter_context(tc.tile_pool(name="rbp", bufs=2))
        outs = ctx.enter_context(tc.tile_pool(name="outs", bufs=2))

        warm = persist.tile([128, 256], BF)

        def _phases():
            # Pre-warm the ACT "exp" table set while the input DMAs stream:
            # otherwise the first real exp pays the ~2.7us table load in the
            # middle of the kernel.
            nc.vector.memset(warm, 0.0)
            nc.scalar.activation(
                warm[:, 0:16], warm[:, 0:16], mybir.ActivationFunctionType.Exp
            )
            # PE warm-up: dummy matmuls on zeros while the first input DMAs
            # land, so the HAM activity window starts ramping the PE clock
            # before the real projection stream begins (output never read).
            wps = oacc.tile([128, 512], F32, tag="oacc", name="wps")
            for _ in range(40):
                nc.tensor.matmul(
                    wps[:, 0:256], warm[:, 0:128], warm, start=True, stop=True
                )

            # ---------------- Phase 1: projections + RoPE ----------------
            def rope(jb, scp, pt):
                xs = ropet.tile([128, 2, 512], BF, tag="xs")
                if jb in (2, 3):
                    # last Q units: keep the ACT queue clear so attention's
                    # first exp isn't stuck behind these copies
                    nc.vector.tensor_copy(xs, pt)
                else:
                    nc.scalar.copy(xs, pt)
                xw = ropet.tile([128, 2, 512], BF, tag="xw")
                nc.sync.dma_start(out=xw[0:64, :, :], in_=xs[64:128, :, :])
                nc.sync.dma_start(out=xw[64:128, :, :], in_=xs[0:64, :, :])
                csl = slice(2 * scp, 2 * scp + 2)
                nc.vector.tensor_mul(xs, xs, cos_sb[:, csl, :])
                nc.vector.tensor_mul(xw, xw, sin_sb[:, csl, :])
                nc.vector.tensor_add(qk_sb[:, jb, csl, :], xs, xw)

            def jsl_of(jb):
                if jb < HQ:
                    return wqs, slice(jb * 128, (jb + 1) * 128)
                kvl = jb - HQ
                return wks, slice(kvl * 128, (kvl + 1) * 128)

            def do_qk(jb):
                w_sb, jsl = jsl_of(jb)
                for scp in range(2):      # pairs of 512-wide s-chunks
                    pt = pairs.tile([128, 2, 512], F32, tag="pairs")
                    for c in range(NE):
                        lhsT = w_sb[:, c, jsl]
                        for k in range(2):
                            sck = 2 * scp + k
                            nc.tensor.matmul(
                                pt[:, k, :], lhsT,
                                xt_sb[:, c, sck * 512:(sck + 1) * 512],
                                start=(c == 0), stop=(c == NE - 1),
                            )
                    rope(jb, scp, pt)

            def do_qk_v(jb, vsts):
                # chunk-major: the qk unit and its paired V columns consume
                # each xT chunk together, keeping PE ahead of the DMA feed
                # during the initial load window
                w_sb, jsl = jsl_of(jb)
                pt0 = pairs.tile([128, 2, 512], F32, tag="pairs", name=f"pt0_{jb}")
                pt1 = pairs.tile([128, 2, 512], F32, tag="pairs", name=f"pt1_{jb}")
                pvs = []
                for i, st in enumerate(vsts):
                    pool = accp if i < 2 else oacc
                    pvs.append(
                        pool.tile([128, 512], F32,
                                  tag="accp" if i < 2 else "oacc",
                                  name=f"pv_{jb}_{st}")
                    )
                # V matmuls lead the qk matmuls by LEAD chunks: at unit
                # boundaries the first qk matmul waits for the previous
                # unit's rope copies to release the scores psum slots, and
                # the leading V matmuls (own psum pool) fill that latency
                LEAD = 5
                for c in range(NE + LEAD):
                    if c < NE:
                        for i, st in enumerate(vsts):
                            nc.tensor.matmul(
                                pvs[i][:, 0:HKV * HD],
                                xt_sb[:, c, st * 128:(st + 1) * 128],
                                wvs[:, c, :],
                                start=(c == 0), stop=(c == NE - 1),
                            )
                    if c >= LEAD:
                        cq = c - LEAD
                        lhsT = w_sb[:, cq, jsl]
                        for scp, pt in enumerate((pt0, pt1)):
                            for k in range(2):
                                sck = 2 * scp + k
                                nc.tensor.matmul(
                                    pt[:, k, :], lhsT,
                                    xt_sb[:, cq, sck * 512:(sck + 1) * 512],
                                    start=(cq == 0), stop=(cq == NE - 1),
                                )
                rope(jb, 0, pt0)
                rope(jb, 1, pt1)
                for i, st in enumerate(vsts):
                    nc.scalar.copy(v_sb[:, st, :], pvs[i][:, 0:HKV * HD])

            def do_v(sts):
                for st in sts:
                    pv = accp.tile([128, 512], F32, tag="accp")
                    for c in range(NE):
                        nc.tensor.matmul(
                            pv[:, 0:HKV * HD],
                            xt_sb[:, c, st * 128:(st + 1) * 128],
                            wvs[:, c, :],
                            start=(c == 0), stop=(c == NE - 1),
                        )
                    nc.scalar.copy(v_sb[:, st, :], pv[:, 0:HKV * HD])

            # K first (attention h=0 needs it), each early unit dragging 3 V
            # columns chunk-major through the DMA feed window
            do_qk_v(HQ, [0, 1, 2])
            do_qk_v(HQ + 1, [3, 4, 5])
            do_qk_v(0, [6, 7, 8])
            do_qk_v(1, [9, 10, 11])
            do_v([12, 13, 14, 15])
            do_qk(2)
            do_qk(3)

            # ---------- Phase 2+3: attention + output projection ----------
            # software-pipelined: outproj(sc-1) is EMITTED after
            # attention(sc) so the scheduler prefers the ACT-gated attention
            # stream and uses outproj matmuls/copies as PE/DVE fill work.
            def attention(sc):
                for h in range(HQ):
                    kvjb = HQ + h // 2
                    kvl = h // 2
                    cps = accp.tile([128, 512], F32, tag="accp")
                    dacc = dccp.tile([128, 512], BF, tag="dacc")

                    def scores(g):
                        sp = pairs.tile([128, 2, 512], F32, tag="pairs")
                        for k in range(2):
                            tcn = 2 * g + k
                            nc.tensor.matmul(
                                sp[:, k, :],
                                qk_sb[:, kvjb, tcn // 4, (tcn % 4) * 128:(tcn % 4) * 128 + 128],
                                qk_sb[:, h, sc, :],
                                start=True, stop=True,
                            )
                        return sp

                    # scores run one pair ahead of exp/ctx so the static PE
                    # stream never blocks on the activation latency
                    sp_next = scores(0)
                    for g in range(8):        # pairs of 128-wide t-chunks
                        sp = sp_next
                        if g < 7:
                            sp_next = scores(g + 1)
                        ex = expool.tile([128, 2, 512], BF, tag="ex")
                        nc.scalar.activation(
                            ex, sp, mybir.ActivationFunctionType.Exp, scale=SCALE,
                        )
                        for k in range(2):
                            nc.tensor.matmul(
                                cps,
                                v_sb[:, 2 * g + k, kvl * 128:(kvl + 1) * 128],
                                ex[:, k, :],
                                start=(g == 0 and k == 0), stop=(g == 7 and k == 1),
                            )
                        if g == 0:
                            nc.vector.tensor_add(dacc, ex[:, 0, :], ex[:, 1, :])
                        else:
                            nc.vector.tensor_add(dacc, dacc, ex[:, 0, :])
                            nc.vector.tensor_add(dacc, dacc, ex[:, 1, :])
                    dar = darp.tile([128, 512], F32, tag="dar")
                    nc.gpsimd.partition_all_reduce(
                        dar, dacc, 128, bass_isa.ReduceOp.add
                    )
                    rb = rbp.tile([128, 512], F32, tag="rb")
                    nc.vector.reciprocal_approx_fast(rb, dar)
                    nc.vector.tensor_mul(ctx_sb[:, h, sc, :], cps, rb)
            # output projection for the 4 s-row-chunks of one sc block.
            # On the last block attention is finished: the scores psum
            # pool and the ACT engine are free, so use them for extra
            # pipeline depth there.
            def outproj(sc):
                last = sc == SC4 - 1
                for so4 in range(4):
                    tail = last and so4 == 3
                    ot4 = outs.tile([128, SC4, 512], BF, tag="ot")
                    for ec in range(SC4):
                        if last and ec % 2 == 0:
                            opsP = pairs.tile([128, 2, 512], F32, tag="pairs")
                        if last:
                            ops = opsP[:, ec % 2, :]
                        else:
                            ops = oacc.tile([128, 512], F32, tag="oacc")
                        for hl in range(HQ):
                            nc.tensor.matmul(
                                ops,
                                ctx_sb[:, hl, sc, so4 * 128:(so4 + 1) * 128],
                                wo_sb[:, hl, ec, :],
                                start=(hl == 0), stop=(hl == HQ - 1),
                            )
                        if ec % 2 == 1:
                            nc.scalar.copy(ot4[:, ec, :], ops)
                        else:
                            nc.vector.tensor_copy(ot4[:, ec, :], ops)
                        if tail:
                            # final row-block: per-chunk DMAs so the kernel's
                            # last semaphore rides a quarter-size transfer
                            so = sc * 4 + so4
                            nc.sync.dma_start(
                                out=out[so * 128:(so + 1) * 128,
                                        ec * 512:(ec + 1) * 512],
                                in_=ot4[:, ec, :],
                            )
                    if not tail:
                        so = sc * 4 + so4
                        nc.sync.dma_start(
                            out=out[so * 128:(so + 1) * 128, :].rearrange(
                                "p (ec e) -> p ec e", e=512
                            ),
                            in_=ot4,
                        )

            attention(0)
            for sc in range(1, SC4):
                attention(sc)
                outproj(sc - 1)
            outproj(SC4 - 1)

        if loop_n is not None:
            with tc.For_i(0, loop_n, 1):
                _phases()
        else:
            _phases()

    nc.compile()
    return nc


def _get_nc():
    global _NC
    if _NC is None:
        _NC = _build_program()
    return _NC


def _rope_tables():
    half = HD // 2
    inv_freq = 1.0 / (10000.0 ** (np.arange(half, dtype=np.float64) * 2.0 / HD))
    ang = np.arange(S, dtype=np.float64)[:, None] * inv_freq[None, :]  # (S, 64)
    cos = np.concatenate([np.cos(ang), np.cos(ang)], axis=1).T  # (128, S)
    sin = np.concatenate([-np.sin(ang), np.sin(ang)], axis=1).T  # pre-signed
    return (np.ascontiguousarray(cos).astype(bfnp),
            np.ascontiguousarray(sin).astype(bfnp))


def build_in_maps(x, W_Q, W_K, W_V, W_O):
    x = np.asarray(x, dtype=np.float32)
    W_Q = np.asarray(W_Q, dtype=np.float32)
    W_K = np.asarray(W_K, dtype=np.float32)
    W_V = np.asarray(W_V, dtype=np.float32)
    W_O = np.asarray(W_O, dtype=np.float32)
    cos, sin = _rope_tables()
    in_maps = []
    xTb = [np.ascontiguousarray(x[b].T).astype(bfnp) for b in range(B)]
    for b in range(B):
        for t in range(TP):
            qheads = list(range(HQ * t, HQ * t + HQ))
            kvheads = [HKV * t + i for i in range(HKV)]
            idxq = [d * HEADS + h for h in qheads for d in range(HD)]
            idxkv = [d * KV + kv for kv in kvheads for d in range(HD)]
            rows_o = [h * HD + d for h in qheads for d in range(HD)]
            in_maps.append(dict(
                xT=xTb[b],
                wq=np.ascontiguousarray(W_Q[idxq, :].T).astype(bfnp),
                wk=np.ascontiguousarray(W_K[idxkv, :].T).astype(bfnp),
                wv=np.ascontiguousarray(W_V[idxkv, :].T).astype(bfnp),
                wo=np.ascontiguousarray(W_O[:, rows_o].T).astype(bfnp),
                cosT=cos,
                sinT=sin,
            ))
    return in_maps


def emulate_core(m):
    """Numpy emulation of the device math for one core's in_map."""
    xT = np.asarray(m["xT"], np.float32)      # (E, S)
    wq = np.asarray(m["wq"], np.float32)      # (E, 512)
    wk = np.asarray(m["wk"], np.float32)
    wv = np.asarray(m["wv"], np.float32)
    wo = np.asarray(m["wo"], np.float32)      # (512, E)
    cos = np.asarray(m["cosT"], np.float32)   # (128, S)
    sin = np.asarray(m["sinT"], np.float32)

    def bfq(a):
        return a.astype(bfnp).astype(np.float32)

    qT = bfq(wq.T @ xT)                       # (512, S)
    kT = bfq(wk.T @ xT)
    vT = bfq(wv.T @ xT)

    def rope(blkT):  # (128, S)
        xw = np.concatenate([blkT[64:], blkT[:64]], axis=0)
        return bfq(blkT * cos + xw * sin)

    ctxs = []
    for h in range(HQ):
        qh = rope(qT[h * 128:(h + 1) * 128])
        kvl = h // 2
        kh = rope(kT[kvl * 128:(kvl + 1) * 128])
        vh = vT[kvl * 128:(kvl + 1) * 128]
        scoresT = kh.T @ qh * SCALE           # (t, s)
        w = bfq(np.exp(scoresT))
        den = w.sum(axis=0)
        ctxT = bfq((vh @ w) / den[None, :])
        ctxs.append(ctxT)
    ctx = np.concatenate(ctxs, axis=0)        # (512, S)
    return bfq(ctx.T @ wo)


def combine_outs(outs):
    out = np.empty((B, S, EMB), dtype=np.float32)
    for b in range(B):
        acc = np.asarray(outs[TP * b]).astype(np.float32)
        for t in range(1, TP):
            acc = acc + np.asarray(outs[TP * b + t]).astype(np.float32)
        out[b] = acc
    return out


LAST_RESULTS = None


def kernel(x, W_Q, W_K, W_V, W_O):
    global LAST_RESULTS
    from concourse.bass_utils import run_bass_kernel_spmd

    nc = _get_nc()
    in_maps = build_in_maps(x, W_Q, W_K, W_V, W_O)
    res = run_bass_kernel_spmd(nc, in_maps, list(range(NCORES)))
    LAST_RESULTS = res
    outs = [r["out"] for r in res.results]
    return combine_outs(outs)


# revision 12
# speedup vs baseline: 2.6855x; 2.6855x over previous
# GQA attention block on 8 Trainium2 NeuronCores — restructured bf16 edition.
# Sharding: core = (batch b in {0,1}) x (tensor-parallel t in {0..3}).
# Each core: batch row b, 4 query heads {4t..4t+3}, 2 kv heads {2t, 2t+1}.
# W_Q/W_K/W_V split column-wise (per-head), W_O row-wise; the 4 TP partial
# outputs per batch are summed on the host (the "all-reduce").
#
# vs the naive schedule:
#  - softmax exp batched over PSUM bank-pairs (N=1024 per ACT instruction)
#  - softmax denominator moved off the tensor engine: DVE running adds over
#    the ex tiles + one gpsimd partition_all_reduce (PE saves a full second
#    pass over ex), reciprocal via the fast DVE approximation
#  - output projection interleaved per 512-row block so PE never drains
#  - RoPE in bf16 (2x DVE modes), output written as bf16 (halves out DMA)
import math
import sys

sys.path.insert(0, "/opt/trn_rl_repo")

import ml_dtypes
import numpy as np

import concourse.bacc as bacc
import concourse.bass as bass
import concourse.bass_isa as bass_isa
import concourse.mybir as mybir
import concourse.tile as tile
from contextlib import ExitStack

BF = mybir.dt.bfloat16
F32 = mybir.dt.float32
bfnp = ml_dtypes.bfloat16

EMB = 2048
HEADS = 16
G = 2
HD = 128          # head dim
KV = HEADS // G   # 8 kv heads
B = 2
S = 2048
NCORES = 8
TP = 4
HQ = HEADS // TP       # 4 q heads per core
HKV = KV // TP         # 2 kv heads per core
NE = EMB // 128        # 16 contraction chunks
SC4 = S // 512         # 4 s-chunks of 512
SC16 = S // 128        # 16 s-chunks of 128
SCALE = 1.0 / math.sqrt(float(EMB))

_NC = None


def _build_program(loop_n=None):
    nc = bacc.Bacc("TRN2", target_bir_lowering=False, debug=False)

    xT = nc.dram_tensor("xT", (EMB, S), BF, kind="ExternalInput")
    wq = nc.dram_tensor("wq", (EMB, HQ * HD), BF, kind="ExternalInput")
    wk = nc.dram_tensor("wk", (EMB, HKV * HD), BF, kind="ExternalInput")
    wv = nc.dram_tensor("wv", (EMB, HKV * HD), BF, kind="ExternalInput")
    wo = nc.dram_tensor("wo", (HQ * HD, EMB), BF, kind="ExternalInput")
    cosT = nc.dram_tensor("cosT", (HD, S), BF, kind="ExternalInput")
    sinT = nc.dram_tensor("sinT", (HD, S), BF, kind="ExternalInput")
    out = nc.dram_tensor("out", (S, EMB), BF, kind="ExternalOutput")

    with tile.TileContext(nc) as tc, ExitStack() as ctx:
        persist = ctx.enter_context(tc.tile_pool(name="persist", bufs=1))
        # roped Q (jb 0..3) and K (jb 4..5), bf16: [d, jb, sc, s512]
        qk_sb = persist.tile([128, HQ + HKV, SC4, 512], BF)
        # V in [t, d] layout: [t_part, t_chunk, kvl*128+d]
        v_sb = persist.tile([128, SC16, HKV * HD], BF)
        ctx_sb = persist.tile([128, HQ, SC4, 512], BF)   # [d, head, sc, s]
        wo_sb = persist.tile([128, HQ, SC4, 512], BF)    # [d, head, ec, e]
        xt_sb = persist.tile([128, NE, S], BF)
        wqs = persist.tile([128, NE, HQ * HD], BF)
        wks = persist.tile([128, NE, HKV * HD], BF)
        wvs = persist.tile([128, NE, HKV * HD], BF)
        cos_sb = persist.tile([128, SC4, 512], BF)
        sin_sb = persist.tile([128, SC4, 512], BF)

        # batched input loads: few multi-dim DMAs (the SP sequencer pays
        # ~0.6us dispatch per DMA). xT is split so its completion semaphores
        # fire progressively and the first projection can start early.
        xTr = xT.rearrange("(c p) s -> p c s", p=128)
        wkr = wk.rearrange("(c p) j -> p c j", p=128)
        nc.sync.dma_start(out=xt_sb[:, 0:1, :], in_=xTr[:, 0:1, :])
        nc.sync.dma_start(out=wks[:, 0:4, :], in_=wkr[:, 0:4, :])
        nc.sync.dma_start(out=xt_sb[:, 1:2, :], in_=xTr[:, 1:2, :])
        nc.sync.dma_start(out=wks[:, 4:8, :], in_=wkr[:, 4:8, :])
        nc.sync.dma_start(out=wks[:, 8:16, :], in_=wkr[:, 8:16, :])
        nc.sync.dma_start(out=xt_sb[:, 2:4, :], in_=xTr[:, 2:4, :])
        for ci in range(2, 6):
            nc.sync.dma_start(
                out=xt_sb[:, 2 * ci:2 * ci + 2, :], in_=xTr[:, 2 * ci:2 * ci + 2, :]
            )
        nc.sync.dma_start(out=wvs, in_=wv.rearrange("(c p) j -> p c j", p=128))
        for ci in range(6, 8):
            nc.sync.dma_start(
                out=xt_sb[:, 2 * ci:2 * ci + 2, :], in_=xTr[:, 2 * ci:2 * ci + 2, :]
            )
        nc.sync.dma_start(out=wqs, in_=wq.rearrange("(c p) j -> p c j", p=128))
        nc.sync.dma_start(out=cos_sb, in_=cosT.rearrange("p (sc s) -> p sc s", s=512))
        nc.sync.dma_start(out=sin_sb, in_=sinT.rearrange("p (sc s) -> p sc s", s=512))
        nc.sync.dma_start(
            out=wo_sb, in_=wo.rearrange("(jb p) (ec e) -> p jb ec e", p=128, e=512)
        )

        # PSUM budget (8 banks): pairs 2x2 + accp 2 + oacc 2
        pairs = ctx.enter_context(tc.tile_pool(name="pairs", bufs=2, space="PSUM"))
        accp = ctx.enter_context(tc.tile_pool(name="accp", bufs=2, space="PSUM"))
        oacc = ctx.enter_context(tc.tile_pool(name="oacc", bufs=2, space="PSUM"))
        ropet = ctx.enter_context(tc.tile_pool(name="ropet", bufs=3))
        expool = ctx.enter_context(tc.tile_pool(name="expool", bufs=3))
        dccp = ctx.enter_context(tc.tile_pool(name="dccp", bufs=2))
        darp = ctx.enter_context(tc.tile_pool(name="darp", bufs=2))
        rbp = ctx.enter_context(tc.tile_pool(name="rbp", bufs=1))
        outs = ctx.enter_context(tc.tile_pool(name="outs", bufs=3))

        warm = persist.tile([128, 256], BF)

        def _phases():
            # Pre-warm the ACT "exp" table set while the input DMAs stream:
            # otherwise the first real exp pays the ~2.7us table load in the
            # middle of the kernel.
            nc.vector.memset(warm, 0.0)
            nc.scalar.activation(
                warm[:, 0:16], warm[:, 0:16], mybir.ActivationFunctionType.Exp
            )
            # PE warm-up: dummy matmuls on zeros while the first input DMAs
            # land, so the HAM activity window starts ramping the PE clock
            # before the real projection stream begins (output never read).
            wps = oacc.tile([128, 512], F32, tag="oacc", name="wps")
            for _ in range(40):
                nc.tensor.matmul(
                    wps[:, 0:256], warm[:, 0:128], warm, start=True, stop=True
                )

            # ---------------- Phase 1: projections + RoPE ----------------
            def rope(jb, scp, pt):
                xs = ropet.tile([128, 2, 512], BF, tag="xs")
                if jb in (2, 3):
                    # last Q units: keep the ACT queue clear so attention's
                    # first exp isn't stuck behind these copies
                    nc.vector.tensor_copy(xs, pt)
                else:
                    nc.scalar.copy(xs, pt)
                xw = ropet.tile([128, 2, 512], BF, tag="xw")
                nc.sync.dma_start(out=xw[0:64, :, :], in_=xs[64:128, :, :])
                nc.sync.dma_start(out=xw[64:128, :, :], in_=xs[0:64, :, :])
                csl = slice(2 * scp, 2 * scp + 2)
                nc.vector.tensor_mul(xs, xs, cos_sb[:, csl, :])
                nc.vector.tensor_mul(xw, xw, sin_sb[:, csl, :])
                nc.vector.tensor_add(qk_sb[:, jb, csl, :], xs, xw)

            def jsl_of(jb):
                if jb < HQ:
                    return wqs, slice(jb * 128, (jb + 1) * 128)
                kvl = jb - HQ
                return wks, slice(kvl * 128, (kvl + 1) * 128)

            def do_qk(jb):
                w_sb, jsl = jsl_of(jb)
                for scp in range(2):      # pairs of 512-wide s-chunks
                    pt = pairs.tile([128, 2, 512], F32, tag="pairs")
                    for c in range(NE):
                        lhsT = w_sb[:, c, jsl]
                        for k in range(2):
                            sck = 2 * scp + k
                            nc.tensor.matmul(
                                pt[:, k, :], lhsT,
                                xt_sb[:, c, sck * 512:(sck + 1) * 512],
                                start=(c == 0), stop=(c == NE - 1),
                            )
                    rope(jb, scp, pt)

            def do_qk_v(jb, vsts):
                # chunk-major: the qk unit and its paired V columns consume
                # each xT chunk together, keeping PE ahead of the DMA feed
                # during the initial load window
                w_sb, jsl = jsl_of(jb)
                pt0 = pairs.tile([128, 2, 512], F32, tag="pairs", name=f"pt0_{jb}")
                pt1 = pairs.tile([128, 2, 512], F32, tag="pairs", name=f"pt1_{jb}")
                pvs = []
                for i, st in enumerate(vsts):
                    pool = accp if i < 2 else oacc
                    pvs.append(
                        pool.tile([128, 512], F32,
                                  tag="accp" if i < 2 else "oacc",
                                  name=f"pv_{jb}_{st}")
                    )
                # V matmuls lead the qk matmuls by LEAD chunks: at unit
                # boundaries the first qk matmul waits for the previous
                # unit's rope copies to release the scores psum slots, and
                # the leading V matmuls (own psum pool) fill that latency
                LEAD = 5
                for c in range(NE + LEAD):
                    if c < NE:
                        for i, st in enumerate(vsts):
                            nc.tensor.matmul(
                                pvs[i][:, 0:HKV * HD],
                                xt_sb[:, c, st * 128:(st + 1) * 128],
                                wvs[:, c, :],
                                start=(c == 0), stop=(c == NE - 1),
                            )
                    if c >= LEAD:
                        cq = c - LEAD
                        lhsT = w_sb[:, cq, jsl]
                        for scp, pt in enumerate((pt0, pt1)):
                            for k in range(2):
                                sck = 2 * scp + k
                                nc.tensor.matmul(
                                    pt[:, k, :], lhsT,
                                    xt_sb[:, cq, sck * 512:(sck + 1) * 512],
                                    start=(cq == 0), stop=(cq == NE - 1),
                                )
                rope(jb, 0, pt0)
                rope(jb, 1, pt1)
                for i, st in enumerate(vsts):
                    nc.scalar.copy(v_sb[:, st, :], pvs[i][:, 0:HKV * HD])

            def do_v(sts):
                for st in sts:
                    pv = accp.tile([128, 512], F32, tag="accp")
                    for c in range(NE):
                        nc.tensor.matmul(
                            pv[:, 0:HKV * HD],
                            xt_sb[:, c, st * 128:(st + 1) * 128],
                            wvs[:, c, :],
                            start=(c == 0), stop=(c == NE - 1),
                        )
                    nc.scalar.copy(v_sb[:, st, :], pv[:, 0:HKV * HD])

            # K first (attention h=0 needs it), each early unit dragging 3 V
            # columns chunk-major through the DMA feed window
            do_qk_v(HQ, [0, 1, 2])
            do_qk_v(HQ + 1, [3, 4, 5])
            do_qk_v(0, [6, 7, 8])
            do_qk_v(1, [9, 10, 11])
            do_v([12, 13, 14, 15])
            do_qk(2)
            do_qk(3)

            # ---------- Phase 2+3: attention + output projection ----------
            # software-pipelined: outproj(sc-1) is EMITTED after
            # attention(sc) so the scheduler prefers the ACT-gated attention
            # stream and uses outproj matmuls/copies as PE/DVE fill work.
            def attention(sc):
                for h in range(HQ):
                    kvjb = HQ + h // 2
                    kvl = h // 2
                    cps = accp.tile([128, 512], F32, tag="accp")
                    dacc = dccp.tile([128, 512], BF, tag="dacc")

                    def scores(g):
                        sp = pairs.tile([128, 2, 512], F32, tag="pairs")
                        for k in range(2):
                            tcn = 2 * g + k
                            nc.tensor.matmul(
                                sp[:, k, :],
                                qk_sb[:, kvjb, tcn // 4, (tcn % 4) * 128:(tcn % 4) * 128 + 128],
                                qk_sb[:, h, sc, :],
                                start=True, stop=True,
                            )
                        return sp

                    # scores run one pair ahead of exp/ctx so the static PE
                    # stream never blocks on the activation latency
                    sp_next = scores(0)
                    for g in range(8):        # pairs of 128-wide t-chunks
                        sp = sp_next
                        if g < 7:
                            sp_next = scores(g + 1)
                        ex = expool.tile([128, 2, 512], BF, tag="ex")
                        nc.scalar.activation(
                            ex, sp, mybir.ActivationFunctionType.Exp, scale=SCALE,
                        )
                        for k in range(2):
                            nc.tensor.matmul(
                                cps,
                                v_sb[:, 2 * g + k, kvl * 128:(kvl + 1) * 128],
                                ex[:, k, :],
                                start=(g == 0 and k == 0), stop=(g == 7 and k == 1),
                            )
                        if g == 0:
                            nc.vector.tensor_add(dacc, ex[:, 0, :], ex[:, 1, :])
                        else:
                            nc.vector.tensor_add(dacc, dacc, ex[:, 0, :])
                            nc.vector.tensor_add(dacc, dacc, ex[:, 1, :])
                    dar = darp.tile([128, 512], F32, tag="dar")
                    nc.gpsimd.partition_all_reduce(
                        dar, dacc, 128, bass_isa.ReduceOp.add
                    )
                    rb = rbp.tile([128, 512], F32, tag="rb")
                    nc.vector.reciprocal_approx_fast(rb, dar)
                    nc.vector.tensor_mul(ctx_sb[:, h, sc, :], cps, rb)
            # output projection for the 4 s-row-chunks of one sc block.
            # On the last block attention is finished: the scores psum
            # pool and the ACT engine are free, so use them for extra
            # pipeline depth there.
            def outproj(sc):
                last = sc == SC4 - 1
                for so4 in range(4):
                    tail = last and so4 == 3
                    ot4 = outs.tile([128, SC4, 512], BF, tag="ot")
                    for ec in range(SC4):
                        if last and ec % 2 == 0:
                            opsP = pairs.tile([128, 2, 512], F32, tag="pairs")
                        if last:
                            ops = opsP[:, ec % 2, :]
                        else:
                            ops = oacc.tile([128, 512], F32, tag="oacc")
                        for hl in range(HQ):
                            nc.tensor.matmul(
                                ops,
                                ctx_sb[:, hl, sc, so4 * 128:(so4 + 1) * 128],
                                wo_sb[:, hl, ec, :],
                                start=(hl == 0), stop=(hl == HQ - 1),
                            )
                        if ec % 2 == 1:
                            nc.scalar.copy(ot4[:, ec, :], ops)
                        else:
                            nc.vector.tensor_copy(ot4[:, ec, :], ops)
                        if tail:
                            # final row-block: per-chunk DMAs so the kernel's
                            # last semaphore rides a quarter-size transfer
                            so = sc * 4 + so4
                            nc.sync.dma_start(
                                out=out[so * 128:(so + 1) * 128,
                                        ec * 512:(ec + 1) * 512],
                                in_=ot4[:, ec, :],
                            )
                    if not tail:
                        so = sc * 4 + so4
                        nc.sync.dma_start(
                            out=out[so * 128:(so + 1) * 128, :].rearrange(
                                "p (ec e) -> p ec e", e=512
                            ),
                            in_=ot4,
                        )

            attention(0)
            for sc in range(1, SC4):
                attention(sc)
                outproj(sc - 1)
            outproj(SC4 - 1)

        if loop_n is not None:
            with tc.For_i(0, loop_n, 1):
                _phases()
        else:
            _phases()

    nc.compile()
    return nc


def _get_nc():
    global _NC
    if _NC is None:
        _NC = _build_program()
    return _NC


def _rope_tables():
    half = HD // 2
    inv_freq = 1.0 / (10000.0 ** (np.arange(half, dtype=np.float64) * 2.0 / HD))
    ang = np.arange(S, dtype=np.float64)[:, None] * inv_freq[None, :]  # (S, 64)
    cos = np.concatenate([np.cos(ang), np.cos(ang)], axis=1).T  # (128, S)
    sin = np.concatenate([-np.sin(ang), np.sin(ang)], axis=1).T  # pre-signed
    return (np.ascontiguousarray(cos).astype(bfnp),
            np.ascontiguousarray(sin).astype(bfnp))


def build_in_maps(x, W_Q, W_K, W_V, W_O):
    x = np.asarray(x, dtype=np.float32)
    W_Q = np.asarray(W_Q, dtype=np.float32)
    W_K = np.asarray(W_K, dtype=np.float32)
    W_V = np.asarray(W_V, dtype=np.float32)
    W_O = np.asarray(W_O, dtype=np.float32)
    cos, sin = _rope_tables()
    in_maps = []
    xTb = [np.ascontiguousarray(x[b].T).astype(bfnp) for b in range(B)]
    for b in range(B):
        for t in range(TP):
            qheads = list(range(HQ * t, HQ * t + HQ))
            kvheads = [HKV * t + i for i in range(HKV)]
            idxq = [d * HEADS + h for h in qheads for d in range(HD)]
            idxkv = [d * KV + kv for kv in kvheads for d in range(HD)]
            rows_o = [h * HD + d for h in qheads for d in range(HD)]
            in_maps.append(dict(
                xT=xTb[b],
                wq=np.ascontiguousarray(W_Q[idxq, :].T).astype(bfnp),
                wk=np.ascontiguousarray(W_K[idxkv, :].T).astype(bfnp),
                wv=np.ascontiguousarray(W_V[idxkv, :].T).astype(bfnp),
                wo=np.ascontiguousarray(W_O[:, rows_o].T).astype(bfnp),
                cosT=cos,
                sinT=sin,
            ))
    return in_maps


def emulate_core(m):
    """Numpy emulation of the device math for one core's in_map."""
    xT = np.asarray(m["xT"], np.float32)      # (E, S)
    wq = np.asarray(m["wq"], np.float32)      # (E, 512)
    wk = np.asarray(m["wk"], np.float32)
    wv = np.asarray(m["wv"], np.float32)
    wo = np.asarray(m["wo"], np.float32)      # (512, E)
    cos = np.asarray(m["cosT"], np.float32)   # (128, S)
    sin = np.asarray(m["sinT"], np.float32)

    def bfq(a):
        return a.astype(bfnp).astype(np.float32)

    qT = bfq(wq.T @ xT)                       # (512, S)
    kT = bfq(wk.T @ xT)
    vT = bfq(wv.T @ xT)

    def rope(blkT):  # (128, S)
        xw = np.concatenate([blkT[64:], blkT[:64]], axis=0)
        return bfq(blkT * cos + xw * sin)

    ctxs = []
    for h in range(HQ):
        qh = rope(qT[h * 128:(h + 1) * 128])
        kvl = h // 2
        kh = rope(kT[kvl * 128:(kvl + 1) * 128])
        vh = vT[kvl * 128:(kvl + 1) * 128]
        scoresT = kh.T @ qh * SCALE           # (t, s)
        w = bfq(np.exp(scoresT))
        den = w.sum(axis=0)
        ctxT = bfq((vh @ w) / den[None, :])
        ctxs.append(ctxT)
    ctx = np.concatenate(ctxs, axis=0)        # (512, S)
    return bfq(ctx.T @ wo)


def combine_outs(outs):
    out = np.empty((B, S, EMB), dtype=np.float32)
    for b in range(B):
        acc = np.asarray(outs[TP * b]).astype(np.float32)
        for t in range(1, TP):
            acc = acc + np.asarray(outs[TP * b + t]).astype(np.float32)
        out[b] = acc
    return out


LAST_RESULTS = None


def kernel(x, W_Q, W_K, W_V, W_O):
    global LAST_RESULTS
    from concourse.bass_utils import run_bass_kernel_spmd

    nc = _get_nc()
    in_maps = build_in_maps(x, W_Q, W_K, W_V, W_O)
    res = run_bass_kernel_spmd(nc, in_maps, list(range(NCORES)))
    LAST_RESULTS = res
    outs = [r["out"] for r in res.results]
    return combine_outs(outs)
